# revision 30
# baseline (speedup 1.0000x reference)
"""Trainium2 Bass kernel for nn_MidAttnBlock (res-block -> full LxL attention -> res-block).

Contract: kernel(**inputs) takes the FULL inputs of reference.setup_inputs()
(x: (16,256,2048) f32, t: (16,256,1) f32, plus conv/groupnorm/linear params)
and returns the FULL (16,256,2048) f32 output.  Data-parallel over batch on
8 NeuronCores, 2 samples per core; each core runs an identical Bass program.

Precision plan (validated against a CPU simulation, total rel err ~6e-3):
 - convs + kqv projection run in bf16 (full PE rate, half SBUF/DMA);
 - the attention core (scores, softmax numerator matmuls, AV) runs in
   fp8e4m3 with DoubleRow perf mode: each matmul contracts K=256 at the
   same moving rate, halving PE time for the L x L phase;
 - exp uses a constant offset (exp(s/16 - 3)) so softmax numerators fit
   fp8 range; the offset cancels in the normalization.

Scheduling: the two samples are software-pipelined at phase granularity so
the PE never drains during groupnorm barriers; groupnorm uses a single
"group-average" matmul (per-channel broadcast stats) placed mid-stream of
the other sample's conv matmuls.

Self-contained: all shapes/sharding hardcoded.
"""

import json as _json

import ml_dtypes
import numpy as np

import concourse.bass as bass
import concourse.bass2jax as _b2j
import concourse.bass_utils as _bu
import concourse.tile as tile
from concourse import mybir
from concourse.vector_clock import ScopedClock, VectorClock


def _split_bir_waits(bir_json):
    """The walrus_driver in this container encodes at most ONE sync-wait per
    instruction (and none on Drain).  Tile's sem assigner attaches several.
    Rewrite the BIR: excess waits move to single-wait NoOps inserted directly
    before the instruction on the same engine."""
    m = _json.loads(bir_json)
    ctr = 0
    for fn in m.get("functions", []):
        for bb in fn.get("blocks", []):
            out = []
            for ins in bb.get("instructions", []):
                si = ins.get("sync_info")
                waits = (si or {}).get("on_wait") or []
                keep = 0 if ins.get("opcode") == "Drain" else 1
                if len(waits) > keep:
                    nmove = len(waits) - keep
                    for w in waits[:nmove]:
                        ctr += 1
                        out.append({
                            "debug": ins.get("debug", 0),
                            "engine": ins["engine"],
                            "ins": [],
                            "name": f"{ins['name']}-wsp{ctr}",
                            "opcode": "NoOp",
                            "outs": [],
                            "sync_info": {"on_update": [], "on_wait": [w]},
                        })
                    si["on_wait"] = waits[nmove:]
                out.append(ins)
            bb["instructions"] = out
    return _json.dumps(m).encode()


_orig_compile_bir_kernel = _bu.compile_bir_kernel


def _compile_bir_splitwaits(bir_json, tmpdir, neff_name="file.neff"):
    return _orig_compile_bir_kernel(_split_bir_waits(bir_json), tmpdir, neff_name)


if getattr(_bu.compile_bir_kernel, "__name__", "") != "_compile_bir_splitwaits":
    _bu.compile_bir_kernel = _compile_bir_splitwaits
    _b2j.compile_bir_kernel = _compile_bir_splitwaits


F32 = mybir.dt.float32
F32R = mybir.dt.float32r
BF16 = mybir.dt.bfloat16
FP8 = mybir.dt.float8e4
AF = mybir.ActivationFunctionType
OP = mybir.AluOpType
DR = mybir.MatmulPerfMode.DoubleRow

P = 128          # partitions
C = 256          # channels
CB = 2           # channel blocks of 128
L = 2048         # sequence length
LS = 512         # l-slice (matmul moving dim)
NL = L // LS     # 4 slices
KB = L // P      # 16 k-blocks for attention
NPAIR = KB // 2  # 8 DoubleRow k-block pairs
EPS = 1e-5
S = 2            # samples per core
NCORES = 8
SCALE = 1.0 / 16.0   # 1/sqrt(C)
EXPOFF = -3.0        # exp(s*SCALE + EXPOFF): keeps softmax numerators in fp8 range


class _TileContextPatched(tile.TileContext):
    """TileContext whose kernel-tail drain carries no sem waits (the container
    walrus rejects waits on Drain); one SP NOP per proc carries them instead."""

    def _drain_and_barrier(self, tick_clock, wait_clock):
        gc = tick_clock.global_clock
        n = len(gc)
        for p in range(n):
            v = gc[p]
            if v > 0:
                vec = [0] * n
                vec[p] = v
                nop = self.nc.sync.nop()
                wait_clock.add_sem_waits(nop.ins, ScopedClock({None: VectorClock(vec)}))
        self.nc.sync.drain()
        self.nc.all_engine_barrier()
        assert self.sems is not None
        popped = self.nc._tile_sem_poison_stack.pop()
        assert popped is self._sem_poison
        self.nc.clear_and_free_semaphores(list(self.sems.allocated().values()))
        self.nc.all_engine_barrier()


def build_program(samples=S, use_bias=()):
    """Build the per-core Bass program (identical on all cores).

    use_bias: subset of {"c2b_r1", "c2b_r2", "linb"} enabling extra adds for
    biases that setup_inputs() keeps at zero.
    """
    nc = bass.Bass()

    # ---- DRAM I/O (per core) ----
    x_d = nc.dram_tensor("x", (samples, C, L), BF16, kind="ExternalInput")
    # t + conv1 bias, host-packed [samples, P, CB, 2(resblock)]
    t_d = nc.dram_tensor("tv", (samples, P, CB, 2), F32, kind="ExternalInput")
    w_conv = {}
    for rb in ("r1", "r2"):
        # host-packed [P(ic within block), icb, tap, oc]
        w_conv[rb, 1] = nc.dram_tensor(f"{rb}_w1t", (P, CB, 3, C), BF16, kind="ExternalInput")
        w_conv[rb, 2] = nc.dram_tensor(f"{rb}_w2t", (P, CB, 3, C), BF16, kind="ExternalInput")
    wkqv_d = nc.dram_tensor("wkqvt", (P, CB, 3 * C), BF16, kind="ExternalInput")
    gnw_d = {}
    for rb in ("r1", "r2"):
        for ln in (1, 2):
            gnw_d[rb, ln, "w"] = nc.dram_tensor(f"{rb}_gn{ln}_ws", (P, CB), F32, kind="ExternalInput")
            gnw_d[rb, ln, "b"] = nc.dram_tensor(f"{rb}_gn{ln}_bs", (P, CB), F32, kind="ExternalInput")
    c2b_d = {}
    if "c2b_r1" in use_bias:
        c2b_d["r1"] = nc.dram_tensor("r1_c2bs", (P, CB), F32, kind="ExternalInput")
    if "c2b_r2" in use_bias:
        c2b_d["r2"] = nc.dram_tensor("r2_c2bs", (P, CB), F32, kind="ExternalInput")
    linb_d = None
    if "linb" in use_bias:
        linb_d = nc.dram_tensor("lin_bs", (P, 3 * CB), F32, kind="ExternalInput")
    gavg_d = nc.dram_tensor("gavg", (P, P), F32R, kind="ExternalInput")   # 1/8 group-avg (block diag)
    onesr_d = nc.dram_tensor("onesr", (1, P), F32R, kind="ExternalInput")
    ones8_d = nc.dram_tensor("ones8", (P, 2, 32), FP8, kind="ExternalInput")
    out_d = nc.dram_tensor("out", (samples, C, L), F32, kind="ExternalOutput")

    from contextlib import ExitStack

    with ExitStack() as _ctx:
        tc = _ctx.enter_context(_TileContextPatched(nc))
        _p = lambda **kw: _ctx.enter_context(tc.tile_pool(**kw))
        consts = _p(name="consts", bufs=1)
        padpA = _p(name="padpA", bufs=3)
        padpB = _p(name="padpB", bufs=3)
        actpA = _p(name="actpA", bufs=2)
        actpB = _p(name="actpB", bufs=2)
        seqpA = _p(name="seqpA", bufs=1)
        seqpB = _p(name="seqpB", bufs=1)
        expp = _p(name="expp", bufs=2)
        outp = _p(name="outp", bufs=3)
        rdbp = _p(name="rdbp", bufs=2)
        smallA = _p(name="smallA", bufs=2)
        smallB = _p(name="smallB", bufs=2)
        t2p = _p(name="t2p", bufs=1)
        pacc = _p(name="pacc", bufs=3, space="PSUM")
        psavp = _p(name="psavp", bufs=2, space="PSUM")
        pdn = _p(name="pdn", bufs=1, space="PSUM")
        prdb = _p(name="prdb", bufs=1, space="PSUM")
        pgn = _p(name="pgn", bufs=1, space="PSUM")

        padp = {0: padpA, 1: padpB}
        actp = {0: actpA, 1: actpB}
        seqp = {0: seqpA, 1: seqpB}
        small = {0: smallA, 1: smallB}

        # ---- persistent constants / weights in SBUF ----
        # conv/lin weights: issued on the scalar queue AFTER the x loads
        # (load_weights() is called right after load(A)/load(B) below)
        w1_sb, w2_sb = {}, {}
        w1_sb["r1"] = consts.tile([P, CB, 3, C], BF16, tag="w1_r1", name="w1_r1")
        w2_sb["r1"] = consts.tile([P, CB, 3, C], BF16, tag="w2_r1", name="w2_r1")
        wkqv_sb = consts.tile([P, CB, 3 * C], BF16, tag="wkqv", name="wkqv")
        w1_sb["r2"] = consts.tile([P, CB, 3, C], BF16, tag="w1_r2", name="w1_r2")
        w2_sb["r2"] = consts.tile([P, CB, 3, C], BF16, tag="w2_r2", name="w2_r2")

        def load_weights():
            nc.scalar.dma_start(w1_sb["r1"][:], w_conv["r1", 1][:])
            nc.scalar.dma_start(w2_sb["r1"][:], w_conv["r1", 2][:])
            nc.scalar.dma_start(wkqv_sb[:], wkqv_d[:])
            nc.scalar.dma_start(w1_sb["r2"][:], w_conv["r2", 1][:])
            nc.scalar.dma_start(w2_sb["r2"][:], w_conv["r2", 2][:])
        # gpsimd queue: small constants
        gavg_sb = consts.tile([P, P], F32R, tag="gavg", name="gavg")
        nc.gpsimd.dma_start(gavg_sb[:], gavg_d[:])
        onesr_sb = consts.tile([1, P], F32R, tag="onesr", name="onesr")
        nc.gpsimd.dma_start(onesr_sb[:], onesr_d[:])
        ones8_sb = consts.tile([P, 2, 32], FP8, tag="ones8", name="ones8")
        nc.gpsimd.dma_start(ones8_sb[:], ones8_d[:])
        gnp_sb = {}
        for rb in ("r1", "r2"):
            for ln in (1, 2):
                for wb in ("w", "b"):
                    tl = consts.tile([P, CB], F32, tag=f"gn_{rb}{ln}{wb}", name=f"gn_{rb}{ln}{wb}")
                    nc.gpsimd.dma_start(tl[:], gnw_d[rb, ln, wb][:])
                    gnp_sb[rb, ln, wb] = tl
        c2b_sb = {}
        for rb, d in c2b_d.items():
            c2b_sb[rb] = consts.tile([P, CB], F32, tag=f"c2b_{rb}", name=f"c2b_{rb}")
            nc.gpsimd.dma_start(c2b_sb[rb][:], d[:])
        linb_sb = None
        if linb_d is not None:
            linb_sb = consts.tile([P, 3 * CB], F32, tag="linb", name="linb")
            nc.gpsimd.dma_start(linb_sb[:], linb_d[:])
        t2_sb = {}
        for s in range(samples):
            t2_sb[s] = t2p.tile([P, CB, 2], F32, tag=f"t2_{s}", name=f"t2_{s}")
            nc.gpsimd.dma_start(t2_sb[s][:], t_d[s])
        eps_sb = consts.tile([P, 1], F32, tag="eps", name="eps")
        nc.vector.memset(eps_sb[:], EPS)
        expoff_sb = consts.tile([P, 1], F32, tag="expoff", name="expoff")
        nc.vector.memset(expoff_sb[:], EXPOFF)
        zeroe = consts.tile([P, 1], BF16, tag="zeroe", name="zeroe")
        nc.vector.memset(zeroe[:], 0.0)

        def alloc_padded(s, tag, pool):
            """[P, L+2] bf16 tile per channel block; data cols [1, L+1), zero edges."""
            ts = []
            for cb in range(CB):
                tl = pool.tile([P, L + 2], BF16, tag=f"{tag}{cb}", name=f"s{s}{tag}{cb}")
                nc.vector.tensor_copy(out=tl[:, 0:1], in_=zeroe[:])
                nc.vector.tensor_copy(out=tl[:, L + 1 : L + 2], in_=zeroe[:])
                ts.append(tl)
            return ts

        # ---- per-sample state ----
        st = [dict() for _ in range(samples)]

        def load(s):
            # the scalar (Act HW-DGE) queue is by far the fastest; the sync (SP)
            # queue services DMA only between its semaphore duties and lags the
            # first ~15us.  x(A) gates the whole pipeline start: give it the
            # scalar queue + gpsimd; x(B) (needed ~10us later) rides sync+gpsimd.
            qs_map = {
                0: [nc.scalar, nc.scalar, nc.scalar, nc.gpsimd, nc.scalar, nc.scalar, nc.gpsimd, nc.gpsimd],
                1: [nc.sync, nc.sync, nc.gpsimd, nc.sync, nc.gpsimd, nc.sync, nc.gpsimd, nc.gpsimd],
            }[s]
            with nc.named_scope(f"s{s}_load"):
                xp = alloc_padded(s, "pad", padp[s])
                for cb in range(CB):
                    for i in range(NL):
                        qs_map[cb * NL + i].dma_start(
                            xp[cb][:, 1 + i * LS : 1 + (i + 1) * LS],
                            x_d[s, cb * P : (cb + 1) * P, i * LS : (i + 1) * LS],
                        )
                st[s]["xp"] = xp

        def stats_alloc(s):
            """[P, CB, NL, 6] stats tile for the next gn of stream s."""
            stt = small[s].tile([P, CB, NL, 6], F32, tag="stats", name=f"s{s}stats")
            st[s]["stats"] = stt
            return stt

        def emit_stats(s, cb, i, src_slice):
            nc.vector.bn_stats(out=st[s]["stats"][:, cb, i, :], in_=src_slice)

        def gn_stats_direct(s, src):
            """Issue bn_stats for all chunks of padded src (used for gn1-of-r1
            where the producer is the input DMA)."""
            stats_alloc(s)
            for cb in range(CB):
                for i in range(NL):
                    emit_stats(s, cb, i, src[cb][:, 1 + i * LS : 1 + (i + 1) * LS])

        def gn_mid(s, rb, ln):
            """Aggregate stats -> per-channel scale/shift.  One PE matmul per
            channel block (group-average broadcast); everything else on
            DVE/ScalarE.  Returns sb tile [P, CB, 2] = (scale, shift)."""
            stt = st[s]["stats"]
            sb = small[s].tile([P, CB, 2], F32, tag="sb", name=f"s{s}sb")
            for cb in range(CB):
                mv = small[s].tile([P, 2], F32, tag="mv", name=f"s{s}mv")
                nc.vector.bn_aggr(out=mv[:], in_=stt[:, cb, :, :])
                # tmp = [mean_c, E[x^2]_c] (f32r: feeds the group-avg matmul)
                tmp = small[s].tile([P, 2], F32R, tag="tmp", name=f"s{s}tmp")
                nc.vector.tensor_copy(out=tmp[:, 0:1], in_=mv[:, 0:1])
                nc.vector.tensor_tensor(out=tmp[:, 1:2], in0=mv[:, 0:1], in1=mv[:, 0:1], op=OP.mult)
                nc.vector.tensor_tensor(out=tmp[:, 1:2], in0=tmp[:, 1:2].bitcast(F32), in1=mv[:, 1:2], op=OP.add)
                bc2 = pgn.tile([P, 2], F32, tag="gn", name=f"s{s}bc2")
                nc.tensor.matmul(bc2[:], gavg_sb[:], tmp[:], start=True, stop=True)
                bcs = small[s].tile([P, 2], F32, tag="bcs", name=f"s{s}bcs")
                nc.vector.tensor_copy(out=bcs[:], in_=bc2[:])
                # var_g = E[x^2]_g - m_g^2 ; rstd = exp(-0.5*ln(var+eps))
                var = small[s].tile([P, 1], F32, tag="var", name=f"s{s}var")
                nc.vector.tensor_tensor(out=var[:], in0=bcs[:, 0:1], in1=bcs[:, 0:1], op=OP.mult)
                nc.vector.tensor_tensor(out=var[:], in0=bcs[:, 1:2], in1=var[:], op=OP.subtract)
                nc.scalar.activation(out=var[:], in_=var[:], func=AF.Ln, bias=eps_sb[:])
                rstd = small[s].tile([P, 1], F32, tag="rstd", name=f"s{s}rstd")
                nc.scalar.activation(out=rstd[:], in_=var[:], func=AF.Exp, scale=-0.5)
                # s = rstd*w ; shift = b - m*s
                nc.vector.tensor_scalar_mul(sb[:, cb, 0:1], rstd[:], gnp_sb[rb, ln, "w"][:, cb : cb + 1])
                ms = small[s].tile([P, 1], F32, tag="ms", name=f"s{s}ms")
                nc.vector.tensor_tensor(out=ms[:], in0=bcs[:, 0:1], in1=sb[:, cb, 0:1], op=OP.mult)
                nc.vector.tensor_scalar_mul(ms[:], ms[:], -1.0)
                nc.vector.tensor_scalar_add(sb[:, cb, 1:2], ms[:], gnp_sb[rb, ln, "b"][:, cb : cb + 1])
            return sb

        def gn_apply(s, src, dst, sb):
            """dst = relu(src*scale + shift) on ScalarE, ls-outer so the
            consuming conv can start after the first pair of chunks."""
            for i in range(NL):
                for cb in range(CB):
                    nc.scalar.activation(
                        out=dst[cb][:, 1 + i * LS : 1 + (i + 1) * LS],
                        in_=src[cb][:, 1 + i * LS : 1 + (i + 1) * LS],
                        func=AF.Relu,
                        bias=sb[:, cb, 1:2],
                        scale=sb[:, cb, 0:1],
                    )

        def conv3(s, src, wt, consume, mid_hook=None, consume_hook=None):
            """3-tap conv over padded bf16 src; consume(ocb, ls, psum_tile).
            mid_hook() is issued between the two ocb streams (a slot where
            cross-stream gn aggregation matmuls cost the PE nothing);
            consume_hook(k) after each consume (cross-stream DVE filler)."""
            for ocb in range(CB):
                for ls in range(NL):
                    ps = pacc.tile([P, LS], F32, tag="acc", name="acc")
                    k = 0
                    for icb in range(CB):
                        for tap in range(3):
                            nc.tensor.matmul(
                                ps[:],
                                wt[:, icb, tap, ocb * P : (ocb + 1) * P],
                                src[icb][:, ls * LS + tap : ls * LS + tap + LS],
                                start=(k == 0),
                                stop=(k == 5),
                            )
                            k += 1
                    consume(ocb, ls, ps)
                    if consume_hook is not None:
                        consume_hook(ocb * NL + ls)
                if ocb == 0 and mid_hook is not None:
                    mid_hook()

        def conv1(s, rb, rbi, src, collect_stats, mid_hook=None, consume_hook=None):
            """h = conv(src) + t; h bf16 padded; optionally emits bn_stats."""
            h = alloc_padded(s, "pad", padp[s])
            if collect_stats:
                stats_alloc(s)
            with nc.named_scope(f"s{s}_{rb}_conv1"):
                def eat1(ocb, ls, ps):
                    dsl = h[ocb][:, 1 + ls * LS : 1 + (ls + 1) * LS]
                    nc.vector.tensor_scalar_add(dsl, ps[:], t2_sb[s][:, ocb, rbi : rbi + 1])
                    if collect_stats:
                        emit_stats(s, ocb, ls, dsl)
                conv3(s, src, w1_sb[rb], eat1, mid_hook, consume_hook)
            return h

        def conv2(s, rb, resid, src, final, mid_hook=None):
            """out = resid + conv(src).  final=False -> bf16 [P, L] pair;
            final=True -> stream f32 to DRAM."""
            res = None
            if not final:
                res = [seqp[s].tile([P, L], BF16, tag=f"res{cb}", name=f"s{s}res{cb}") for cb in range(CB)]
            with nc.named_scope(f"s{s}_{rb}_conv2"):
                def eat2(ocb, ls, ps):
                    if rb in c2b_sb:
                        nc.vector.tensor_scalar_add(ps[:], ps[:], c2b_sb[rb][:, ocb : ocb + 1])
                    rsl = resid[ocb][:, 1 + ls * LS : 1 + (ls + 1) * LS]
                    if final:
                        ot = outp.tile([P, LS], F32, tag="out", name="ot")
                        nc.vector.tensor_tensor(out=ot[:], in0=ps[:], in1=rsl, op=OP.add)
                        (nc.sync if s == 0 else nc.scalar).dma_start(
                            out_d[s, ocb * P : (ocb + 1) * P, ls * LS : (ls + 1) * LS], ot[:]
                        )
                    else:
                        nc.vector.tensor_tensor(
                            out=res[ocb][:, ls * LS : (ls + 1) * LS],
                            in0=ps[:], in1=rsl, op=OP.add,
                        )
                conv3(s, src, w2_sb[rb], eat2, mid_hook)
            return res

        def kqv(s):
            """kt/qt: fp8 [P, CB, L] (c-partition); vt: fp8 [P, KB, C] (l-partition)."""
            x1 = st[s]["x1"]
            kt = seqp[s].tile([P, CB, L], FP8, tag="kt", name=f"s{s}kt")
            qt = seqp[s].tile([P, CB, L], FP8, tag="qt", name=f"s{s}qt")
            vt = seqp[s].tile([P, KB, C], FP8, tag="vt", name=f"s{s}vt")
            with nc.named_scope(f"s{s}_kqv"):
                for j, dst in ((0, kt), (1, qt)):
                    for ocb in range(CB):
                        off = j * C + ocb * P
                        for ls in range(NL):
                            ps = pacc.tile([P, LS], F32, tag="acc", name="acc")
                            for icb in range(CB):
                                nc.tensor.matmul(
                                    ps[:],
                                    wkqv_sb[:, icb, off : off + P],
                                    x1[icb][:, ls * LS : (ls + 1) * LS],
                                    start=(icb == 0),
                                    stop=(icb == 1),
                                )
                            dsl = dst[:, ocb, ls * LS : (ls + 1) * LS]
                            if linb_sb is not None:
                                nc.scalar.activation(
                                    out=dsl, in_=ps[:], func=AF.Identity,
                                    bias=linb_sb[:, j * CB + ocb : j * CB + ocb + 1],
                                )
                            elif ls % 2 == 0:
                                nc.scalar.copy(out=dsl, in_=ps[:])
                            else:
                                nc.vector.tensor_copy(out=dsl, in_=ps[:])
                # vT[l, c] (l on partitions) for the attention output matmul
                for lb in range(KB):
                    ps = pacc.tile([P, LS], F32, tag="acc", name="acc")
                    for icb in range(CB):
                        nc.tensor.matmul(
                            ps[:, :C],
                            x1[icb][:, lb * P : (lb + 1) * P],
                            wkqv_sb[:, icb, 2 * C : 3 * C],
                            start=(icb == 0),
                            stop=(icb == 1),
                        )
                    if lb % 2 == 0:
                        nc.vector.tensor_copy(out=vt[:, lb, :], in_=ps[:, :C])
                    else:
                        nc.scalar.copy(out=vt[:, lb, :], in_=ps[:, :C])
            st[s]["kt"], st[s]["qt"], st[s]["vt"] = kt, qt, vt

        def attn(s, pre_hooks=()):
            """Full-L attention with fp8 DoubleRow matmuls.  psav/dn drain to
            SBUF unnormalized right away; the reciprocal+broadcast+scale bundle
            (part2: PE+DVE only, never ScalarE -- ScalarE's in-order FIFO must
            stay pure exp during attention) is deferred into the next qs's
            matmul stream.  pre_hooks fire one per qs at the same j==4 slot and
            must also be ScalarE-free unless their inputs are long ready.
            Returns a closure flushing the last qs's part2."""
            kt, qt, vt = st[s]["kt"], st[s]["qt"], st[s]["vt"]
            av = alloc_padded(s, "pad", padp[s])
            stats_alloc(s)
            hooks = list(pre_hooks)
            box = [None]

            def flush():
                if box[0] is not None:
                    box[0]()
                    box[0] = None

            for qs in range(NL):
                with nc.named_scope(f"s{s}_attn{qs}"):
                    dn = pdn.tile([32, LS], F32, tag="dn", name="dn")
                    psav = [psavp.tile([P, LS], F32, tag="psav", name="psav") for _ in range(CB)]
                    ex = expp.tile([P, KB, LS], FP8, tag="exp", name="exp")
                    qsl = qt[:, :, qs * LS : (qs + 1) * LS]
                    for j in range(NPAIR):
                        pss = []
                        for h in range(2):
                            kbg = 2 * j + h
                            ps = pacc.tile([P, LS], F32, tag="acc", name="acc")
                            nc.tensor.matmul(
                                ps[:],
                                kt[:, :, kbg * P : (kbg + 1) * P],
                                qsl,
                                start=True, stop=True, perf_mode=DR,
                            )
                            pss.append(ps)
                        for h in range(2):
                            nc.scalar.activation(
                                out=ex[:, 2 * j + h, :], in_=pss[h][:],
                                func=AF.Exp, scale=SCALE, bias=expoff_sb[:],
                            )
                        exsl = ex[:, 2 * j : 2 * j + 2, :]
                        nc.tensor.matmul(
                            dn[:], ones8_sb[:], exsl,
                            start=(j == 0), stop=(j == NPAIR - 1), perf_mode=DR,
                        )
                        for cb in range(CB):
                            nc.tensor.matmul(
                                psav[cb][:],
                                vt[:, 2 * j : 2 * j + 2, cb * P : (cb + 1) * P],
                                exsl,
                                start=(j == 0), stop=(j == NPAIR - 1), perf_mode=DR,
                            )
                        if j == 4:
                            flush()
                            if hooks:
                                hooks.pop(0)()
                    # drain PSUM immediately (unnormalized); reciprocal runs on
                    # DVE during the next qs's matmuls
                    dns = rdbp.tile([1, LS], F32, tag="dns", name="dns")
                    nc.vector.tensor_copy(out=dns[:], in_=dn[0:1, :])
                    for cb in range(CB):
                        nc.vector.tensor_copy(
                            out=av[cb][:, 1 + qs * LS : 1 + (qs + 1) * LS], in_=psav[cb][:]
                        )
                    rd = rdbp.tile([1, LS], F32R, tag="rd", name="rd")
                    with nc.allow_low_precision("softmax reciprocal feeds f32r broadcast matmul"):
                        nc.vector.reciprocal(out=rd[:], in_=dns[:])

                    def part2(qs=qs, rd=rd):
                        rb_ps = prdb.tile([P, LS], F32, tag="rdb", name="rb_ps")
                        nc.tensor.matmul(rb_ps[:], onesr_sb[:], rd[:], start=True, stop=True)
                        for cb in range(CB):
                            avs = av[cb][:, 1 + qs * LS : 1 + (qs + 1) * LS]
                            nc.vector.tensor_tensor(out=avs, in0=avs, in1=rb_ps[:], op=OP.mult)
                            if linb_sb is not None:
                                nc.vector.tensor_scalar_add(
                                    avs, avs, linb_sb[:, 2 * CB + cb : 2 * CB + cb + 1]
                                )
                            emit_stats(s, cb, qs, avs)
                    box[0] = part2
            st[s]["av"] = av
            return flush

        # ================= interleaved schedule (A=0, B=1) =================
        A, B = 0, 1
        load(A); load(B)
        load_weights()

        # r1 gn1(A) now; gn1(B)'s bn_stats are interleaved into conv1(A)'s
        # consume stream (so they sit behind, not ahead of, A's DVE work) and
        # its aggregation+apply issue right after conv1(A).
        gn_stats_direct(A, st[A]["xp"])
        sbA = gn_mid(A, "r1", 1)
        aA = alloc_padded(A, "act", actp[A])
        gn_apply(A, st[A]["xp"], aA, sbA)
        aB = alloc_padded(B, "act", actp[B])

        def gmid_apply(s, rb, ln, src, dst):
            sb = gn_mid(s, rb, ln)
            gn_apply(s, src, dst, sb)

        st[B]["stats"] = small[B].tile([P, CB, NL, 6], F32, tag="stats", name="s1stats")

        def statsB_hook(k):
            cb, i = k // NL, k % NL
            emit_stats(B, cb, i, st[B]["xp"][cb][:, 1 + i * LS : 1 + (i + 1) * LS])

        hA = conv1(A, "r1", 0, aA, collect_stats=True, consume_hook=statsB_hook)
        gmid_apply(B, "r1", 1, st[B]["xp"], aB)

        a2A = alloc_padded(A, "act", actp[A])
        hB = conv1(B, "r1", 0, aB, collect_stats=True,
                   mid_hook=lambda: gmid_apply(A, "r1", 2, hA, a2A))

        a2B = alloc_padded(B, "act", actp[B])
        st[A]["x1"] = conv2(A, "r1", st[A]["xp"], a2A, final=False,
                            mid_hook=lambda: gmid_apply(B, "r1", 2, hB, a2B))
        st[B]["x1"] = conv2(B, "r1", st[B]["xp"], a2B, final=False)

        kqv(A)
        kqv(B)

        finA = attn(A)
        sbra = {}

        def gmid_attn(s, rb, ln):
            sbra[s] = gn_mid(s, rb, ln)

        finB = attn(B, pre_hooks=[finA, lambda: gmid_attn(A, "r2", 1)])

        raA = alloc_padded(A, "act", actp[A])
        gn_apply(A, st[A]["av"], raA, sbra[A])
        h2A = conv1(A, "r2", 1, raA, collect_stats=True, mid_hook=finB)
        gmid_attn(B, "r2", 1)

        raB = alloc_padded(B, "act", actp[B])
        gn_apply(B, st[B]["av"], raB, sbra[B])
        ra2A = alloc_padded(A, "act", actp[A])
        h2B = conv1(B, "r2", 1, raB, collect_stats=True,
                    mid_hook=lambda: gmid_apply(A, "r2", 2, h2A, ra2A))

        ra2B = alloc_padded(B, "act", actp[B])
        conv2(A, "r2", st[A]["av"], ra2A, final=True,
              mid_hook=lambda: gmid_apply(B, "r2", 2, h2B, ra2B))
        conv2(B, "r2", st[B]["av"], ra2B, final=True)

    nc.finalize()
    return nc


def _pack_conv_w(w):
    """(O, I, 3) -> [P, icb, tap, oc] bf16."""
    w = np.asarray(w, dtype=np.float32)
    o, i, k = w.shape
    return np.ascontiguousarray(
        w.transpose(1, 2, 0).reshape(CB, P, 3, o).transpose(1, 0, 2, 3)
    ).astype(ml_dtypes.bfloat16)


def _pack_gn(v):
    """(256,) -> [P, CB]"""
    return np.ascontiguousarray(np.asarray(v, dtype=np.float32).reshape(CB, P).T)


def make_in_maps(inp, use_bias):
    """Host-side packing; returns the per-core input maps."""
    gavg = np.zeros((P, P), np.float32)
    for cc in range(P):
        g0 = (cc // 8) * 8
        gavg[g0 : g0 + 8, cc] = 0.125
    shared = {
        "wkqvt": np.ascontiguousarray(
            inp["lin_w"][:, :, 0].T.reshape(CB, P, 3 * C).transpose(1, 0, 2)
        ).astype(ml_dtypes.bfloat16),
        "gavg": gavg,
        "onesr": np.ones((1, P), np.float32),
        "ones8": np.ones((P, 2, 32), ml_dtypes.float8_e4m3),
    }
    for rb in ("r1", "r2"):
        shared[f"{rb}_w1t"] = _pack_conv_w(inp[f"{rb}_c1_w"])
        shared[f"{rb}_w2t"] = _pack_conv_w(inp[f"{rb}_c2_w"])
        for ln in (1, 2):
            shared[f"{rb}_gn{ln}_ws"] = _pack_gn(inp[f"{rb}_gn{ln}_w"])
            shared[f"{rb}_gn{ln}_bs"] = _pack_gn(inp[f"{rb}_gn{ln}_b"])
    if "c2b_r1" in use_bias:
        shared["r1_c2bs"] = _pack_gn(inp["r1_c2_b"])
    if "c2b_r2" in use_bias:
        shared["r2_c2bs"] = _pack_gn(inp["r2_c2_b"])
    if "linb" in use_bias:
        shared["lin_bs"] = np.ascontiguousarray(inp["lin_b"].reshape(3 * CB, P).T)

    # per-sample conv1 bias vector: t[s] + c1_b per res block -> [P, CB, 2]
    tfull = inp["t"][:, :, 0]  # (B, C)
    nb = inp["x"].shape[0]
    tv = np.empty((nb, P, CB, 2), np.float32)
    for rbi, rb in enumerate(("r1", "r2")):
        v = tfull + inp[f"{rb}_c1_b"][None, :]
        tv[:, :, :, rbi] = v.reshape(nb, CB, P).transpose(0, 2, 1)

    xb = np.ascontiguousarray(inp["x"]).astype(ml_dtypes.bfloat16)
    in_maps = []
    for c in range(NCORES):
        sl = slice(S * c, S * (c + 1))
        m = dict(shared)
        m["x"] = xb[sl]
        m["tv"] = np.ascontiguousarray(tv[sl])
        in_maps.append(m)
    return in_maps


_CACHE = {}


def kernel(**inputs):
    inp = {k: np.ascontiguousarray(np.asarray(v, dtype=np.float32)) for k, v in inputs.items()}

    use_bias = []
    if np.any(inp["r1_c2_b"]):
        use_bias.append("c2b_r1")
    if np.any(inp["r2_c2_b"]):
        use_bias.append("c2b_r2")
    if np.any(inp["lin_b"]):
        use_bias.append("linb")
    use_bias = tuple(use_bias)

    if ("nc", use_bias) not in _CACHE:
        _CACHE[("nc", use_bias)] = build_program(S, use_bias)
    nc = _CACHE[("nc", use_bias)]

    in_maps = make_in_maps(inp, use_bias)
    res = _bu.run_bass_kernel_spmd(nc, in_maps, core_ids=list(range(NCORES)))
    out = np.concatenate([res.results[c]["out"] for c in range(NCORES)], axis=0)
    return out.astype(np.float32)


# revision 32
# speedup vs baseline: 1.0198x; 1.0198x over previous
"""Trainium2 Bass kernel for nn_MidAttnBlock (res-block -> full LxL attention -> res-block).

Contract: kernel(**inputs) takes the FULL inputs of reference.setup_inputs()
(x: (16,256,2048) f32, t: (16,256,1) f32, plus conv/groupnorm/linear params)
and returns the FULL (16,256,2048) f32 output.  Data-parallel over batch on
8 NeuronCores, 2 samples per core; each core runs an identical Bass program.

Precision plan (validated against a CPU simulation, total rel err ~6e-3):
 - convs + kqv projection run in bf16 (full PE rate, half SBUF/DMA);
 - the attention core (scores, softmax numerator matmuls, AV) runs in
   fp8e4m3 with DoubleRow perf mode: each matmul contracts K=256 at the
   same moving rate, halving PE time for the L x L phase;
 - exp uses a constant offset (exp(s/16 - 3)) so softmax numerators fit
   fp8 range; the offset cancels in the normalization.

Scheduling: the two samples are software-pipelined at phase granularity so
the PE never drains during groupnorm barriers; groupnorm uses a single
"group-average" matmul (per-channel broadcast stats) placed mid-stream of
the other sample's conv matmuls.

Self-contained: all shapes/sharding hardcoded.
"""

import json as _json

import ml_dtypes
import numpy as np

import concourse.bass as bass
import concourse.bass2jax as _b2j
import concourse.bass_utils as _bu
import concourse.tile as tile
from concourse import mybir
from concourse.vector_clock import ScopedClock, VectorClock


def _split_bir_waits(bir_json):
    """The walrus_driver in this container encodes at most ONE sync-wait per
    instruction (and none on Drain).  Tile's sem assigner attaches several.
    Rewrite the BIR: excess waits move to single-wait NoOps inserted directly
    before the instruction on the same engine."""
    m = _json.loads(bir_json)
    ctr = 0
    for fn in m.get("functions", []):
        for bb in fn.get("blocks", []):
            out = []
            for ins in bb.get("instructions", []):
                si = ins.get("sync_info")
                waits = (si or {}).get("on_wait") or []
                keep = 0 if ins.get("opcode") == "Drain" else 1
                if len(waits) > keep:
                    nmove = len(waits) - keep
                    for w in waits[:nmove]:
                        ctr += 1
                        out.append({
                            "debug": ins.get("debug", 0),
                            "engine": ins["engine"],
                            "ins": [],
                            "name": f"{ins['name']}-wsp{ctr}",
                            "opcode": "NoOp",
                            "outs": [],
                            "sync_info": {"on_update": [], "on_wait": [w]},
                        })
                    si["on_wait"] = waits[nmove:]
                out.append(ins)
            bb["instructions"] = out
    return _json.dumps(m).encode()


_orig_compile_bir_kernel = _bu.compile_bir_kernel


def _compile_bir_splitwaits(bir_json, tmpdir, neff_name="file.neff"):
    return _orig_compile_bir_kernel(_split_bir_waits(bir_json), tmpdir, neff_name)


if getattr(_bu.compile_bir_kernel, "__name__", "") != "_compile_bir_splitwaits":
    _bu.compile_bir_kernel = _compile_bir_splitwaits
    _b2j.compile_bir_kernel = _compile_bir_splitwaits


F32 = mybir.dt.float32
F32R = mybir.dt.float32r
BF16 = mybir.dt.bfloat16
FP8 = mybir.dt.float8e4
AF = mybir.ActivationFunctionType
OP = mybir.AluOpType
DR = mybir.MatmulPerfMode.DoubleRow

P = 128          # partitions
C = 256          # channels
CB = 2           # channel blocks of 128
L = 2048         # sequence length
LS = 512         # l-slice (matmul moving dim)
NL = L // LS     # 4 slices
KB = L // P      # 16 k-blocks for attention
NPAIR = KB // 2  # 8 DoubleRow k-block pairs
EPS = 1e-5
S = 2            # samples per core
NCORES = 8
SCALE = 1.0 / 16.0   # 1/sqrt(C)
EXPOFF = -3.0        # exp(s*SCALE + EXPOFF): keeps softmax numerators in fp8 range


class _TileContextPatched(tile.TileContext):
    """TileContext whose kernel-tail drain carries no sem waits (the container
    walrus rejects waits on Drain); one SP NOP per proc carries them instead."""

    def _drain_and_barrier(self, tick_clock, wait_clock):
        gc = tick_clock.global_clock
        n = len(gc)
        for p in range(n):
            v = gc[p]
            if v > 0:
                vec = [0] * n
                vec[p] = v
                nop = self.nc.sync.nop()
                wait_clock.add_sem_waits(nop.ins, ScopedClock({None: VectorClock(vec)}))
        self.nc.sync.drain()
        self.nc.all_engine_barrier()
        assert self.sems is not None
        popped = self.nc._tile_sem_poison_stack.pop()
        assert popped is self._sem_poison
        self.nc.clear_and_free_semaphores(list(self.sems.allocated().values()))
        self.nc.all_engine_barrier()


def build_program(samples=S, use_bias=()):
    """Build the per-core Bass program (identical on all cores).

    use_bias: subset of {"c2b_r1", "c2b_r2", "linb"} enabling extra adds for
    biases that setup_inputs() keeps at zero.
    """
    nc = bass.Bass()

    # ---- DRAM I/O (per core) ----
    x_d = nc.dram_tensor("x", (samples, C, L), BF16, kind="ExternalInput")
    # t + conv1 bias, host-packed [samples, P, CB, 2(resblock)]
    t_d = nc.dram_tensor("tv", (samples, P, CB, 2), F32, kind="ExternalInput")
    w_conv = {}
    for rb in ("r1", "r2"):
        # host-packed [P(ic within block), icb, tap, oc]
        w_conv[rb, 1] = nc.dram_tensor(f"{rb}_w1t", (P, CB, 3, C), BF16, kind="ExternalInput")
        w_conv[rb, 2] = nc.dram_tensor(f"{rb}_w2t", (P, CB, 3, C), BF16, kind="ExternalInput")
    wkqv_d = nc.dram_tensor("wkqvt", (P, CB, 3 * C), BF16, kind="ExternalInput")
    gnw_d = {}
    for rb in ("r1", "r2"):
        for ln in (1, 2):
            gnw_d[rb, ln, "w"] = nc.dram_tensor(f"{rb}_gn{ln}_ws", (P, CB), F32, kind="ExternalInput")
            gnw_d[rb, ln, "b"] = nc.dram_tensor(f"{rb}_gn{ln}_bs", (P, CB), F32, kind="ExternalInput")
    c2b_d = {}
    if "c2b_r1" in use_bias:
        c2b_d["r1"] = nc.dram_tensor("r1_c2bs", (P, CB), F32, kind="ExternalInput")
    if "c2b_r2" in use_bias:
        c2b_d["r2"] = nc.dram_tensor("r2_c2bs", (P, CB), F32, kind="ExternalInput")
    linb_d = None
    if "linb" in use_bias:
        linb_d = nc.dram_tensor("lin_bs", (P, 3 * CB), F32, kind="ExternalInput")
    gavg_d = nc.dram_tensor("gavg", (P, P), F32R, kind="ExternalInput")   # 1/8 group-avg (block diag)
    onesr_d = nc.dram_tensor("onesr", (1, P), F32R, kind="ExternalInput")
    ones8_d = nc.dram_tensor("ones8", (P, 2, 32), FP8, kind="ExternalInput")
    out_d = nc.dram_tensor("out", (samples, C, L), F32, kind="ExternalOutput")

    from contextlib import ExitStack

    with ExitStack() as _ctx:
        tc = _ctx.enter_context(_TileContextPatched(nc))
        _p = lambda **kw: _ctx.enter_context(tc.tile_pool(**kw))
        consts = _p(name="consts", bufs=1)
        padpA = _p(name="padpA", bufs=3)
        padpB = _p(name="padpB", bufs=3)
        actpA = _p(name="actpA", bufs=2)
        actpB = _p(name="actpB", bufs=2)
        seqpA = _p(name="seqpA", bufs=1)
        seqpB = _p(name="seqpB", bufs=1)
        expp = _p(name="expp", bufs=2)
        outp = _p(name="outp", bufs=3)
        rdbp = _p(name="rdbp", bufs=2)
        smallA = _p(name="smallA", bufs=2)
        smallB = _p(name="smallB", bufs=2)
        t2p = _p(name="t2p", bufs=1)
        pacc = _p(name="pacc", bufs=3, space="PSUM")
        psavp = _p(name="psavp", bufs=2, space="PSUM")
        pdn = _p(name="pdn", bufs=1, space="PSUM")
        prdb = _p(name="prdb", bufs=1, space="PSUM")
        pgn = _p(name="pgn", bufs=1, space="PSUM")

        padp = {0: padpA, 1: padpB}
        actp = {0: actpA, 1: actpB}
        seqp = {0: seqpA, 1: seqpB}
        small = {0: smallA, 1: smallB}

        # ---- persistent constants / weights in SBUF ----
        # conv/lin weights: issued on the scalar queue AFTER the x loads
        # (load_weights() is called right after load(A)/load(B) below)
        w1_sb, w2_sb = {}, {}
        w1_sb["r1"] = consts.tile([P, CB, 3, C], BF16, tag="w1_r1", name="w1_r1")
        w2_sb["r1"] = consts.tile([P, CB, 3, C], BF16, tag="w2_r1", name="w2_r1")
        wkqv_sb = consts.tile([P, CB, 3 * C], BF16, tag="wkqv", name="wkqv")
        w1_sb["r2"] = consts.tile([P, CB, 3, C], BF16, tag="w1_r2", name="w1_r2")
        w2_sb["r2"] = consts.tile([P, CB, 3, C], BF16, tag="w2_r2", name="w2_r2")

        def load_weights():
            nc.scalar.dma_start(w1_sb["r1"][:], w_conv["r1", 1][:])
            nc.scalar.dma_start(w2_sb["r1"][:], w_conv["r1", 2][:])
            nc.scalar.dma_start(wkqv_sb[:], wkqv_d[:])
            nc.scalar.dma_start(w1_sb["r2"][:], w_conv["r2", 1][:])
            nc.scalar.dma_start(w2_sb["r2"][:], w_conv["r2", 2][:])
        # gpsimd queue: small constants
        gavg_sb = consts.tile([P, P], F32R, tag="gavg", name="gavg")
        nc.gpsimd.dma_start(gavg_sb[:], gavg_d[:])
        onesr_sb = consts.tile([1, P], F32R, tag="onesr", name="onesr")
        nc.gpsimd.dma_start(onesr_sb[:], onesr_d[:])
        ones8_sb = consts.tile([P, 2, 32], FP8, tag="ones8", name="ones8")
        nc.gpsimd.dma_start(ones8_sb[:], ones8_d[:])
        gnp_sb = {}
        for rb in ("r1", "r2"):
            for ln in (1, 2):
                for wb in ("w", "b"):
                    tl = consts.tile([P, CB], F32, tag=f"gn_{rb}{ln}{wb}", name=f"gn_{rb}{ln}{wb}")
                    nc.gpsimd.dma_start(tl[:], gnw_d[rb, ln, wb][:])
                    gnp_sb[rb, ln, wb] = tl
        c2b_sb = {}
        for rb, d in c2b_d.items():
            c2b_sb[rb] = consts.tile([P, CB], F32, tag=f"c2b_{rb}", name=f"c2b_{rb}")
            nc.gpsimd.dma_start(c2b_sb[rb][:], d[:])
        linb_sb = None
        if linb_d is not None:
            linb_sb = consts.tile([P, 3 * CB], F32, tag="linb", name="linb")
            nc.gpsimd.dma_start(linb_sb[:], linb_d[:])
        t2_sb = {}
        for s in range(samples):
            t2_sb[s] = t2p.tile([P, CB, 2], F32, tag=f"t2_{s}", name=f"t2_{s}")
            nc.gpsimd.dma_start(t2_sb[s][:], t_d[s])
        eps_sb = consts.tile([P, 1], F32, tag="eps", name="eps")
        nc.vector.memset(eps_sb[:], EPS)
        expoff_sb = consts.tile([P, 1], F32, tag="expoff", name="expoff")
        nc.vector.memset(expoff_sb[:], EXPOFF)
        zeroe = consts.tile([P, 1], BF16, tag="zeroe", name="zeroe")
        nc.vector.memset(zeroe[:], 0.0)

        def alloc_padded(s, tag, pool):
            """[P, L+2] bf16 tile per channel block; data cols [1, L+1), zero edges."""
            ts = []
            for cb in range(CB):
                tl = pool.tile([P, L + 2], BF16, tag=f"{tag}{cb}", name=f"s{s}{tag}{cb}")
                nc.vector.tensor_copy(out=tl[:, 0:1], in_=zeroe[:])
                nc.vector.tensor_copy(out=tl[:, L + 1 : L + 2], in_=zeroe[:])
                ts.append(tl)
            return ts

        # ---- per-sample state ----
        st = [dict() for _ in range(samples)]

        def load(s):
            # the scalar (Act HW-DGE) queue is by far the fastest; the sync (SP)
            # queue services DMA only between its semaphore duties and lags the
            # first ~15us.  x(A) gates the whole pipeline start: give it the
            # scalar queue + gpsimd; x(B) (needed ~10us later) rides sync+gpsimd.
            qs_map = {
                0: [nc.scalar, nc.sync, nc.scalar, nc.sync, nc.scalar, nc.sync, nc.scalar, nc.sync],
                1: [nc.sync, nc.sync, nc.sync, nc.sync, nc.gpsimd, nc.sync, nc.gpsimd, nc.gpsimd],
            }[s]
            with nc.named_scope(f"s{s}_load"):
                xp = alloc_padded(s, "pad", padp[s])
                for cb in range(CB):
                    for i in range(NL):
                        qs_map[cb * NL + i].dma_start(
                            xp[cb][:, 1 + i * LS : 1 + (i + 1) * LS],
                            x_d[s, cb * P : (cb + 1) * P, i * LS : (i + 1) * LS],
                        )
                st[s]["xp"] = xp

        def stats_alloc(s):
            """[P, CB, NL, 6] stats tile for the next gn of stream s."""
            stt = small[s].tile([P, CB, NL, 6], F32, tag="stats", name=f"s{s}stats")
            st[s]["stats"] = stt
            return stt

        def emit_stats(s, cb, i, src_slice):
            nc.vector.bn_stats(out=st[s]["stats"][:, cb, i, :], in_=src_slice)

        def gn_stats_direct(s, src):
            """Issue bn_stats for all chunks of padded src (used for gn1-of-r1
            where the producer is the input DMA)."""
            stats_alloc(s)
            for cb in range(CB):
                for i in range(NL):
                    emit_stats(s, cb, i, src[cb][:, 1 + i * LS : 1 + (i + 1) * LS])

        def gn_mid(s, rb, ln):
            """Aggregate stats -> per-channel scale/shift.  One PE matmul per
            channel block (group-average broadcast); everything else on
            DVE/ScalarE.  Returns sb tile [P, CB, 2] = (scale, shift)."""
            stt = st[s]["stats"]
            sb = small[s].tile([P, CB, 2], F32, tag="sb", name=f"s{s}sb")
            for cb in range(CB):
                mv = small[s].tile([P, 2], F32, tag="mv", name=f"s{s}mv")
                nc.vector.bn_aggr(out=mv[:], in_=stt[:, cb, :, :])
                # tmp = [mean_c, E[x^2]_c] (f32r: feeds the group-avg matmul)
                tmp = small[s].tile([P, 2], F32R, tag="tmp", name=f"s{s}tmp")
                nc.vector.tensor_copy(out=tmp[:, 0:1], in_=mv[:, 0:1])
                nc.vector.tensor_tensor(out=tmp[:, 1:2], in0=mv[:, 0:1], in1=mv[:, 0:1], op=OP.mult)
                nc.vector.tensor_tensor(out=tmp[:, 1:2], in0=tmp[:, 1:2].bitcast(F32), in1=mv[:, 1:2], op=OP.add)
                bc2 = pgn.tile([P, 2], F32, tag="gn", name=f"s{s}bc2")
                nc.tensor.matmul(bc2[:], gavg_sb[:], tmp[:], start=True, stop=True)
                bcs = small[s].tile([P, 2], F32, tag="bcs", name=f"s{s}bcs")
                nc.vector.tensor_copy(out=bcs[:], in_=bc2[:])
                # var_g = E[x^2]_g - m_g^2 ; rstd = exp(-0.5*ln(var+eps))
                var = small[s].tile([P, 1], F32, tag="var", name=f"s{s}var")
                nc.vector.tensor_tensor(out=var[:], in0=bcs[:, 0:1], in1=bcs[:, 0:1], op=OP.mult)
                nc.vector.tensor_tensor(out=var[:], in0=bcs[:, 1:2], in1=var[:], op=OP.subtract)
                nc.scalar.activation(out=var[:], in_=var[:], func=AF.Ln, bias=eps_sb[:])
                rstd = small[s].tile([P, 1], F32, tag="rstd", name=f"s{s}rstd")
                nc.scalar.activation(out=rstd[:], in_=var[:], func=AF.Exp, scale=-0.5)
                # s = rstd*w ; shift = b - m*s
                nc.vector.tensor_scalar_mul(sb[:, cb, 0:1], rstd[:], gnp_sb[rb, ln, "w"][:, cb : cb + 1])
                ms = small[s].tile([P, 1], F32, tag="ms", name=f"s{s}ms")
                nc.vector.tensor_tensor(out=ms[:], in0=bcs[:, 0:1], in1=sb[:, cb, 0:1], op=OP.mult)
                nc.vector.tensor_scalar_mul(ms[:], ms[:], -1.0)
                nc.vector.tensor_scalar_add(sb[:, cb, 1:2], ms[:], gnp_sb[rb, ln, "b"][:, cb : cb + 1])
            return sb

        def gn_apply(s, src, dst, sb):
            """dst = relu(src*scale + shift) on ScalarE, ls-outer so the
            consuming conv can start after the first pair of chunks."""
            for i in range(NL):
                for cb in range(CB):
                    nc.scalar.activation(
                        out=dst[cb][:, 1 + i * LS : 1 + (i + 1) * LS],
                        in_=src[cb][:, 1 + i * LS : 1 + (i + 1) * LS],
                        func=AF.Relu,
                        bias=sb[:, cb, 1:2],
                        scale=sb[:, cb, 0:1],
                    )

        def conv3(s, src, wt, consume, mid_hook=None, consume_hook=None):
            """3-tap conv over padded bf16 src; consume(ocb, ls, psum_tile).
            mid_hook() is issued between the two ocb streams (a slot where
            cross-stream gn aggregation matmuls cost the PE nothing);
            consume_hook(k) after each consume (cross-stream DVE filler)."""
            for ocb in range(CB):
                for ls in range(NL):
                    ps = pacc.tile([P, LS], F32, tag="acc", name="acc")
                    k = 0
                    for icb in range(CB):
                        for tap in range(3):
                            nc.tensor.matmul(
                                ps[:],
                                wt[:, icb, tap, ocb * P : (ocb + 1) * P],
                                src[icb][:, ls * LS + tap : ls * LS + tap + LS],
                                start=(k == 0),
                                stop=(k == 5),
                            )
                            k += 1
                    consume(ocb, ls, ps)
                    if consume_hook is not None:
                        consume_hook(ocb * NL + ls)
                if ocb == 0 and mid_hook is not None:
                    mid_hook()

        def conv1(s, rb, rbi, src, collect_stats, mid_hook=None, consume_hook=None):
            """h = conv(src) + t; h bf16 padded; optionally emits bn_stats."""
            h = alloc_padded(s, "pad", padp[s])
            if collect_stats:
                stats_alloc(s)
            with nc.named_scope(f"s{s}_{rb}_conv1"):
                def eat1(ocb, ls, ps):
                    dsl = h[ocb][:, 1 + ls * LS : 1 + (ls + 1) * LS]
                    nc.vector.tensor_scalar_add(dsl, ps[:], t2_sb[s][:, ocb, rbi : rbi + 1])
                    if collect_stats:
                        emit_stats(s, ocb, ls, dsl)
                conv3(s, src, w1_sb[rb], eat1, mid_hook, consume_hook)
            return h

        def conv2(s, rb, resid, src, final, mid_hook=None):
            """out = resid + conv(src).  final=False -> bf16 [P, L] pair;
            final=True -> stream f32 to DRAM."""
            res = None
            if not final:
                res = [seqp[s].tile([P, L], BF16, tag=f"res{cb}", name=f"s{s}res{cb}") for cb in range(CB)]
            with nc.named_scope(f"s{s}_{rb}_conv2"):
                def eat2(ocb, ls, ps):
                    if rb in c2b_sb:
                        nc.vector.tensor_scalar_add(ps[:], ps[:], c2b_sb[rb][:, ocb : ocb + 1])
                    rsl = resid[ocb][:, 1 + ls * LS : 1 + (ls + 1) * LS]
                    if final:
                        ot = outp.tile([P, LS], F32, tag="out", name="ot")
                        nc.vector.tensor_tensor(out=ot[:], in0=ps[:], in1=rsl, op=OP.add)
                        (nc.sync if s == 0 else nc.scalar).dma_start(
                            out_d[s, ocb * P : (ocb + 1) * P, ls * LS : (ls + 1) * LS], ot[:]
                        )
                    else:
                        nc.vector.tensor_tensor(
                            out=res[ocb][:, ls * LS : (ls + 1) * LS],
                            in0=ps[:], in1=rsl, op=OP.add,
                        )
                conv3(s, src, w2_sb[rb], eat2, mid_hook)
            return res

        def kqv(s):
            """kt/qt: fp8 [P, CB, L] (c-partition); vt: fp8 [P, KB, C] (l-partition)."""
            x1 = st[s]["x1"]
            kt = seqp[s].tile([P, CB, L], FP8, tag="kt", name=f"s{s}kt")
            qt = seqp[s].tile([P, CB, L], FP8, tag="qt", name=f"s{s}qt")
            vt = seqp[s].tile([P, KB, C], FP8, tag="vt", name=f"s{s}vt")
            with nc.named_scope(f"s{s}_kqv"):
                for j, dst in ((0, kt), (1, qt)):
                    for ocb in range(CB):
                        off = j * C + ocb * P
                        for ls in range(NL):
                            ps = pacc.tile([P, LS], F32, tag="acc", name="acc")
                            for icb in range(CB):
                                nc.tensor.matmul(
                                    ps[:],
                                    wkqv_sb[:, icb, off : off + P],
                                    x1[icb][:, ls * LS : (ls + 1) * LS],
                                    start=(icb == 0),
                                    stop=(icb == 1),
                                )
                            dsl = dst[:, ocb, ls * LS : (ls + 1) * LS]
                            if linb_sb is not None:
                                nc.scalar.activation(
                                    out=dsl, in_=ps[:], func=AF.Identity,
                                    bias=linb_sb[:, j * CB + ocb : j * CB + ocb + 1],
                                )
                            elif ls % 2 == 0:
                                nc.scalar.copy(out=dsl, in_=ps[:])
                            else:
                                nc.vector.tensor_copy(out=dsl, in_=ps[:])
                # vT[l, c] (l on partitions) for the attention output matmul
                for lb in range(KB):
                    ps = pacc.tile([P, LS], F32, tag="acc", name="acc")
                    for icb in range(CB):
                        nc.tensor.matmul(
                            ps[:, :C],
                            x1[icb][:, lb * P : (lb + 1) * P],
                            wkqv_sb[:, icb, 2 * C : 3 * C],
                            start=(icb == 0),
                            stop=(icb == 1),
                        )
                    if lb % 2 == 0:
                        nc.vector.tensor_copy(out=vt[:, lb, :], in_=ps[:, :C])
                    else:
                        nc.scalar.copy(out=vt[:, lb, :], in_=ps[:, :C])
            st[s]["kt"], st[s]["qt"], st[s]["vt"] = kt, qt, vt

        def attn(s, pre_hooks=()):
            """Full-L attention with fp8 DoubleRow matmuls.  psav/dn drain to
            SBUF unnormalized right away; the reciprocal+broadcast+scale bundle
            (part2: PE+DVE only, never ScalarE -- ScalarE's in-order FIFO must
            stay pure exp during attention) is deferred into the next qs's
            matmul stream.  pre_hooks fire one per qs at the same j==4 slot and
            must also be ScalarE-free unless their inputs are long ready.
            Returns a closure flushing the last qs's part2."""
            kt, qt, vt = st[s]["kt"], st[s]["qt"], st[s]["vt"]
            av = alloc_padded(s, "pad", padp[s])
            stats_alloc(s)
            hooks = list(pre_hooks)
            box = [None]

            def flush():
                if box[0] is not None:
                    box[0]()
                    box[0] = None

            for qs in range(NL):
                with nc.named_scope(f"s{s}_attn{qs}"):
                    dn = pdn.tile([32, LS], F32, tag="dn", name="dn")
                    psav = [psavp.tile([P, LS], F32, tag="psav", name="psav") for _ in range(CB)]
                    ex = expp.tile([P, KB, LS], FP8, tag="exp", name="exp")
                    qsl = qt[:, :, qs * LS : (qs + 1) * LS]
                    for j in range(NPAIR):
                        pss = []
                        for h in range(2):
                            kbg = 2 * j + h
                            ps = pacc.tile([P, LS], F32, tag="acc", name="acc")
                            nc.tensor.matmul(
                                ps[:],
                                kt[:, :, kbg * P : (kbg + 1) * P],
                                qsl,
                                start=True, stop=True, perf_mode=DR,
                            )
                            pss.append(ps)
                        for h in range(2):
                            nc.scalar.activation(
                                out=ex[:, 2 * j + h, :], in_=pss[h][:],
                                func=AF.Exp, scale=SCALE, bias=expoff_sb[:],
                            )
                        exsl = ex[:, 2 * j : 2 * j + 2, :]
                        nc.tensor.matmul(
                            dn[:], ones8_sb[:], exsl,
                            start=(j == 0), stop=(j == NPAIR - 1), perf_mode=DR,
                        )
                        for cb in range(CB):
                            nc.tensor.matmul(
                                psav[cb][:],
                                vt[:, 2 * j : 2 * j + 2, cb * P : (cb + 1) * P],
                                exsl,
                                start=(j == 0), stop=(j == NPAIR - 1), perf_mode=DR,
                            )
                        if j == 4:
                            flush()
                            if hooks:
                                hooks.pop(0)()
                    # drain PSUM immediately (unnormalized); reciprocal runs on
                    # DVE during the next qs's matmuls
                    dns = rdbp.tile([1, LS], F32, tag="dns", name="dns")
                    nc.vector.tensor_copy(out=dns[:], in_=dn[0:1, :])
                    for cb in range(CB):
                        nc.vector.tensor_copy(
                            out=av[cb][:, 1 + qs * LS : 1 + (qs + 1) * LS], in_=psav[cb][:]
                        )
                    rd = rdbp.tile([1, LS], F32R, tag="rd", name="rd")
                    with nc.allow_low_precision("softmax reciprocal feeds f32r broadcast matmul"):
                        nc.vector.reciprocal(out=rd[:], in_=dns[:])

                    def part2(qs=qs, rd=rd):
                        rb_ps = prdb.tile([P, LS], F32, tag="rdb", name="rb_ps")
                        nc.tensor.matmul(rb_ps[:], onesr_sb[:], rd[:], start=True, stop=True)
                        for cb in range(CB):
                            avs = av[cb][:, 1 + qs * LS : 1 + (qs + 1) * LS]
                            nc.vector.tensor_tensor(out=avs, in0=avs, in1=rb_ps[:], op=OP.mult)
                            if linb_sb is not None:
                                nc.vector.tensor_scalar_add(
                                    avs, avs, linb_sb[:, 2 * CB + cb : 2 * CB + cb + 1]
                                )
                            emit_stats(s, cb, qs, avs)
                    box[0] = part2
            st[s]["av"] = av
            return flush

        # ================= interleaved schedule (A=0, B=1) =================
        A, B = 0, 1
        load(A); load(B)
        load_weights()

        # r1 gn1(A) now; gn1(B)'s bn_stats are interleaved into conv1(A)'s
        # consume stream (so they sit behind, not ahead of, A's DVE work) and
        # its aggregation+apply issue right after conv1(A).
        gn_stats_direct(A, st[A]["xp"])
        sbA = gn_mid(A, "r1", 1)
        aA = alloc_padded(A, "act", actp[A])
        gn_apply(A, st[A]["xp"], aA, sbA)
        aB = alloc_padded(B, "act", actp[B])

        def gmid_apply(s, rb, ln, src, dst):
            sb = gn_mid(s, rb, ln)
            gn_apply(s, src, dst, sb)

        st[B]["stats"] = small[B].tile([P, CB, NL, 6], F32, tag="stats", name="s1stats")

        def statsB_hook(k):
            cb, i = k // NL, k % NL
            emit_stats(B, cb, i, st[B]["xp"][cb][:, 1 + i * LS : 1 + (i + 1) * LS])

        hA = conv1(A, "r1", 0, aA, collect_stats=True, consume_hook=statsB_hook)
        gmid_apply(B, "r1", 1, st[B]["xp"], aB)

        a2A = alloc_padded(A, "act", actp[A])
        hB = conv1(B, "r1", 0, aB, collect_stats=True,
                   mid_hook=lambda: gmid_apply(A, "r1", 2, hA, a2A))

        a2B = alloc_padded(B, "act", actp[B])
        st[A]["x1"] = conv2(A, "r1", st[A]["xp"], a2A, final=False,
                            mid_hook=lambda: gmid_apply(B, "r1", 2, hB, a2B))
        st[B]["x1"] = conv2(B, "r1", st[B]["xp"], a2B, final=False)

        kqv(A)
        kqv(B)

        finA = attn(A)
        sbra = {}

        def gmid_attn(s, rb, ln):
            sbra[s] = gn_mid(s, rb, ln)

        finB = attn(B, pre_hooks=[finA, lambda: gmid_attn(A, "r2", 1)])

        raA = alloc_padded(A, "act", actp[A])
        gn_apply(A, st[A]["av"], raA, sbra[A])
        h2A = conv1(A, "r2", 1, raA, collect_stats=True, mid_hook=finB)
        gmid_attn(B, "r2", 1)

        raB = alloc_padded(B, "act", actp[B])
        gn_apply(B, st[B]["av"], raB, sbra[B])
        ra2A = alloc_padded(A, "act", actp[A])
        h2B = conv1(B, "r2", 1, raB, collect_stats=True,
                    mid_hook=lambda: gmid_apply(A, "r2", 2, h2A, ra2A))

        ra2B = alloc_padded(B, "act", actp[B])
        conv2(A, "r2", st[A]["av"], ra2A, final=True,
              mid_hook=lambda: gmid_apply(B, "r2", 2, h2B, ra2B))
        conv2(B, "r2", st[B]["av"], ra2B, final=True)

    nc.finalize()
    return nc


def _pack_conv_w(w):
    """(O, I, 3) -> [P, icb, tap, oc] bf16."""
    w = np.asarray(w, dtype=np.float32)
    o, i, k = w.shape
    return np.ascontiguousarray(
        w.transpose(1, 2, 0).reshape(CB, P, 3, o).transpose(1, 0, 2, 3)
    ).astype(ml_dtypes.bfloat16)


def _pack_gn(v):
    """(256,) -> [P, CB]"""
    return np.ascontiguousarray(np.asarray(v, dtype=np.float32).reshape(CB, P).T)


def make_in_maps(inp, use_bias):
    """Host-side packing; returns the per-core input maps."""
    gavg = np.zeros((P, P), np.float32)
    for cc in range(P):
        g0 = (cc // 8) * 8
        gavg[g0 : g0 + 8, cc] = 0.125
    shared = {
        "wkqvt": np.ascontiguousarray(
            inp["lin_w"][:, :, 0].T.reshape(CB, P, 3 * C).transpose(1, 0, 2)
        ).astype(ml_dtypes.bfloat16),
        "gavg": gavg,
        "onesr": np.ones((1, P), np.float32),
        "ones8": np.ones((P, 2, 32), ml_dtypes.float8_e4m3),
    }
    for rb in ("r1", "r2"):
        shared[f"{rb}_w1t"] = _pack_conv_w(inp[f"{rb}_c1_w"])
        shared[f"{rb}_w2t"] = _pack_conv_w(inp[f"{rb}_c2_w"])
        for ln in (1, 2):
            shared[f"{rb}_gn{ln}_ws"] = _pack_gn(inp[f"{rb}_gn{ln}_w"])
            shared[f"{rb}_gn{ln}_bs"] = _pack_gn(inp[f"{rb}_gn{ln}_b"])
    if "c2b_r1" in use_bias:
        shared["r1_c2bs"] = _pack_gn(inp["r1_c2_b"])
    if "c2b_r2" in use_bias:
        shared["r2_c2bs"] = _pack_gn(inp["r2_c2_b"])
    if "linb" in use_bias:
        shared["lin_bs"] = np.ascontiguousarray(inp["lin_b"].reshape(3 * CB, P).T)

    # per-sample conv1 bias vector: t[s] + c1_b per res block -> [P, CB, 2]
    tfull = inp["t"][:, :, 0]  # (B, C)
    nb = inp["x"].shape[0]
    tv = np.empty((nb, P, CB, 2), np.float32)
    for rbi, rb in enumerate(("r1", "r2")):
        v = tfull + inp[f"{rb}_c1_b"][None, :]
        tv[:, :, :, rbi] = v.reshape(nb, CB, P).transpose(0, 2, 1)

    xb = np.ascontiguousarray(inp["x"]).astype(ml_dtypes.bfloat16)
    in_maps = []
    for c in range(NCORES):
        sl = slice(S * c, S * (c + 1))
        m = dict(shared)
        m["x"] = xb[sl]
        m["tv"] = np.ascontiguousarray(tv[sl])
        in_maps.append(m)
    return in_maps


_CACHE = {}


def kernel(**inputs):
    inp = {k: np.ascontiguousarray(np.asarray(v, dtype=np.float32)) for k, v in inputs.items()}

    use_bias = []
    if np.any(inp["r1_c2_b"]):
        use_bias.append("c2b_r1")
    if np.any(inp["r2_c2_b"]):
        use_bias.append("c2b_r2")
    if np.any(inp["lin_b"]):
        use_bias.append("linb")
    use_bias = tuple(use_bias)

    if ("nc", use_bias) not in _CACHE:
        _CACHE[("nc", use_bias)] = build_program(S, use_bias)
    nc = _CACHE[("nc", use_bias)]

    in_maps = make_in_maps(inp, use_bias)
    res = _bu.run_bass_kernel_spmd(nc, in_maps, core_ids=list(range(NCORES)))
    out = np.concatenate([res.results[c]["out"] for c in range(NCORES)], axis=0)
    return out.astype(np.float32)
